# revision 13
# baseline (speedup 1.0000x reference)
"""Causal self-attention (B=4, T=2048, C=1024, H=16) on 8 TRN2 NeuronCores.

Tensor-parallel over heads: each core owns 2 heads (q/k/v column slice of
c_attn, matching row slice of c_proj) and computes a partial projection
output over the full batch; partials are summed on the host.

Device layout notes (per core):
  - x is staged transposed+bf16 on the host: xt [C, B*T].
  - QKV matmuls produce qT/kT [128, T] per batch (2 heads stacked 64+64 on
    partitions, head_dim on partitions) and a vT staging tile that is
    PE-transposed into token-major v_aug tiles carrying an extra ones
    column (yields the softmax denominator for free during the AV matmul).
  - Scores are computed transposed, S^T [k_tok, q_tok], two heads packed
    into one PE pass via K=64 row-tiling. exp(s/8) runs on ACT straight
    from PSUM into SBUF (no max-subtraction: scores are O(1) here, exp is
    overflow-safe). Causality = trimmed matmul ranges + one static
    triangular mask on diagonal 128-blocks.
  - AV accumulates Y^T in PSUM: head0 as [y(64) | denom] rows 0..64,
    head1 via a [zeros|ones|v] weight layout so its rows land on
    partitions 63..128 (denom row 63) — the two normalized halves form a
    single [128, 512] rhs for the K=128 projection matmul.
  - Normalization: in-place reciprocal of the denom row, DMA
    partition-broadcast through a DRAM bounce, one tensor_mul per head.
  - tri/ident mask constants ship from the host as tiny extra inputs.
Output is the transposed partial projection outp [C, B*T] f32.
"""

import numpy as np
import ml_dtypes

import concourse.bass as bass
import concourse.tile as tile
from concourse import mybir
from concourse.bass_utils import run_bass_kernel_spmd
from concourse.vector_clock import ScopedClock, VectorClock

BF16 = mybir.dt.bfloat16
F32 = mybir.dt.float32
EXPF = mybir.ActivationFunctionType.Exp

B, T, C, H = 4, 2048, 1024, 16
D = C // H          # 64
NCORES = 8
HPC = H // NCORES   # 2 heads per core
DC = HPC * D        # 128 channels per core
SCALE = 1.0 / float(np.sqrt(D))


def _patch_tile_drain():
    """walrus's Drain template rejects >2 sync waits; split the tail-drain
    waits one proc per drain."""
    if getattr(tile.TileContext, "_drain_patched", False):
        return

    def _drain_and_barrier(self, tick_clock, wait_clock):
        nc = self.nc
        gc = tick_clock.global_clock
        n = len(gc)
        for p in range(n):
            if gc[p] > 0:
                d = nc.sync.drain()
                vc = VectorClock([gc[p] if i == p else 0 for i in range(n)])
                wait_clock.add_sem_waits(d.ins, ScopedClock({None: vc}))
        nc.sync.drain()
        nc.all_engine_barrier()
        assert self.sems is not None
        popped = nc._tile_sem_poison_stack.pop()
        assert popped is self._sem_poison
        nc.clear_and_free_semaphores(list(self.sems.allocated().values()))
        nc.all_engine_barrier()

    tile.TileContext._drain_and_barrier = _drain_and_barrier
    tile.TileContext._drain_patched = True


def _split_excess_waits(nc, max_waits=1):
    """walrus's per-instruction sync-wait slot limit is 2; hoist excess
    waits onto same-engine nops inserted just before the instruction."""
    import bass_rust

    snapshots = [
        (bb, list(bb.instructions)) for f in nc.m.functions for bb in f.blocks
    ]
    for bb, insts in snapshots:
        new_list = []
        changed = False
        for inst in insts:
            si = inst.sync_info
            waits = list(si.on_wait) if (si and si.on_wait) else []
            if len(waits) > max_waits:
                changed = True
                excess, keep = waits[:-max_waits], waits[-max_waits:]
                eng = nc.engines[inst.engine]
                for i in range(0, len(excess), max_waits):
                    nop_inst = eng.nop().ins
                    nop_inst.sync_info = bass_rust.SyncInfo(
                        on_wait=list(excess[i:i + max_waits]), on_update=[]
                    )
                    new_list.append(nop_inst)
                inst.sync_info = bass_rust.SyncInfo(
                    on_wait=keep,
                    on_update=list(si.on_update) if si.on_update else [],
                )
            new_list.append(inst)
        bb.instructions = new_list


def _bcast_part(src_row: bass.AP, nparts: int) -> bass.AP:
    """AP reading one partition row broadcast across `nparts` partitions
    (partition stride 0) — used as a DMA source."""
    return bass.AP(
        tensor=src_row.tensor,
        offset=src_row.offset,
        ap=[[0, nparts]] + list(src_row.ap[1:]),
    )


def build_attention_nc(nb: int = B, tb: int = T):
    """One-core program; SPMD across cores via per-core input values."""
    assert tb % 512 == 0
    ntok = nb * tb
    nqc = tb // 512          # q chunks per batch
    nkt = tb // 128          # k tiles per batch
    ncc = C // 128           # contraction chunks for QKV

    nc = bass.Bass("TRN2", target_bir_lowering=False, debug=False)
    xt = nc.dram_tensor("xt", [C, ntok], BF16, kind="ExternalInput").ap()
    wqkv = nc.dram_tensor("wqkv", [C, 3 * DC], BF16, kind="ExternalInput").ap()
    wp = nc.dram_tensor("wp", [DC, C], BF16, kind="ExternalInput").ap()
    trid = nc.dram_tensor("tri", [128, 128], BF16, kind="ExternalInput").ap()
    identd = nc.dram_tensor("ident", [128, 128], BF16, kind="ExternalInput").ap()
    outp = nc.dram_tensor("outp", [C, ntok], F32, kind="ExternalOutput").ap()

    with tile.TileContext(nc) as tc:
        with (
            tc.tile_pool(name="const", bufs=1) as const,
            tc.tile_pool(name="xtp", bufs=2 * ncc) as xtp,
            tc.tile_pool(name="qkp", bufs=2) as qkp,
            tc.tile_pool(name="vap", bufs=2) as vap,
            tc.tile_pool(name="ep", bufs=4) as ep,
            tc.tile_pool(name="nstg", bufs=4) as nstg,
            tc.tile_pool(name="ynp", bufs=2) as ynp,
            tc.tile_pool(name="ostg", bufs=4) as ostg,
            tc.tile_pool(name="drp", bufs=4, space="DRAM") as drp,
            tc.tile_pool(name="mmps", bufs=2, space="PSUM") as mmps,
            tc.tile_pool(name="scps", bufs=2, space="PSUM") as scps,
            tc.tile_pool(name="y0ps", bufs=1, space="PSUM") as y0ps,
            tc.tile_pool(name="y1ps", bufs=1, space="PSUM") as y1ps,
        ):
            # ---- constants ----
            wqkv_sb = const.tile([128, ncc * 3 * DC], BF16)
            for ci in range(ncc):
                nc.sync.dma_start(
                    wqkv_sb[:, ci * 3 * DC:(ci + 1) * 3 * DC],
                    wqkv[128 * ci:128 * (ci + 1), :],
                )
            wp_sb = const.tile([128, C], BF16)
            nc.sync.dma_start(wp_sb, wp)
            ident = const.tile([128, 128], BF16)
            nc.sync.dma_start(ident, identd)
            tri = const.tile([128, 128], BF16)
            nc.sync.dma_start(tri, trid)

            for b in range(nb):
                t0 = b * tb
                # ---- load x^T chunks for this batch ----
                xts = []
                for ci in range(ncc):
                    xtile = xtp.tile([128, tb], BF16)
                    nc.sync.dma_start(
                        xtile, xt[128 * ci:128 * (ci + 1), t0:t0 + tb]
                    )
                    xts.append(xtile)

                # ---- QKV: qT/kT [128, tb], vT staging ----
                qT = qkp.tile([128, tb], BF16, tag="qT")
                kT = qkp.tile([128, tb], BF16, tag="kT")
                vTs = qkp.tile([128, tb], BF16, tag="vTs")
                for oi, dest in ((0, qT), (1, kT), (2, vTs)):
                    for t4 in range(tb // 512):
                        ps = mmps.tile([128, 512], F32, tag="mm")
                        for ci in range(ncc):
                            nc.tensor.matmul(
                                ps,
                                lhsT=wqkv_sb[
                                    :, ci * 3 * DC + oi * DC:
                                    ci * 3 * DC + (oi + 1) * DC
                                ],
                                rhs=xts[ci][:, 512 * t4:512 * (t4 + 1)],
                                start=(ci == 0),
                                stop=(ci == ncc - 1),
                            )
                        nc.vector.tensor_copy(
                            dest[:, 512 * t4:512 * (t4 + 1)], ps
                        )

                # ---- v_aug: token-major v with ones column ----
                # head0: per ktile 66 cols = [v(64) | 1 | pad]
                # head1: per ktile 128 cols = [0(32) | 1 | 0(31) | v(64)]
                va0 = vap.tile([128, nkt * 66], BF16, tag="va0")
                va1 = vap.tile([128, nkt * 128], BF16, tag="va1")
                va0v = va0.rearrange("p (t c) -> p t c", c=66)
                va1v = va1.rearrange("p (t c) -> p t c", c=128)
                nc.vector.memset(va1, 0.0)
                nc.vector.memset(va0v[:, :, 64:66], 0.0)
                nc.vector.memset(va0v[:, :, 64:65], 1.0)
                nc.vector.memset(va1v[:, :, 32:33], 1.0)
                for g in range(nkt // 4):
                    tps = mmps.tile([128, 512], BF16, tag="mm")
                    for j in range(4):
                        nc.tensor.transpose(
                            tps[:, 128 * j:128 * (j + 1)],
                            vTs[:, 128 * (4 * g + j):128 * (4 * g + j + 1)],
                            ident,
                        )
                    tpsv = tps.rearrange("p (t c) -> p t c", c=128)
                    nc.vector.tensor_copy(
                        va0v[:, 4 * g:4 * g + 4, 0:64], tpsv[:, 0:4, 0:64]
                    )
                    nc.vector.tensor_copy(
                        va1v[:, 4 * g:4 * g + 4, 64:128], tpsv[:, 0:4, 64:128]
                    )

                # ---- attention + projection, per 512-token q chunk ----
                for qc in range(nqc):
                    q0 = 512 * qc
                    y0 = y0ps.tile([65, 512], F32)
                    y1 = y1ps.tile([128, 512], F32)
                    nkts = 4 * (qc + 1)
                    for kt in range(nkts):
                        off = max(0, 128 * kt - q0)
                        sc = scps.tile([128, 1024], F32, tag="sc")
                        nc.tensor.matmul(
                            sc[:, off:512],
                            lhsT=kT[0:64, 128 * kt:128 * (kt + 1)],
                            rhs=qT[0:64, q0 + off:q0 + 512],
                            start=True, stop=True,
                        )
                        nc.tensor.matmul(
                            sc[:, 512 + off:1024],
                            lhsT=kT[64:128, 128 * kt:128 * (kt + 1)],
                            rhs=qT[64:128, q0 + off:q0 + 512],
                            start=True, stop=True,
                        )
                        e = ep.tile([128, 1024], BF16)
                        if off == 0:
                            nc.scalar.activation(e, sc, EXPF, scale=SCALE)
                        else:
                            nc.scalar.activation(
                                e[:, off:512], sc[:, off:512], EXPF,
                                scale=SCALE,
                            )
                            nc.scalar.activation(
                                e[:, 512 + off:1024], sc[:, 512 + off:1024],
                                EXPF, scale=SCALE,
                            )
                        if 128 * kt >= q0:  # diagonal block: triangular mask
                            nc.vector.tensor_mul(
                                e[:, off:off + 128], e[:, off:off + 128], tri
                            )
                            nc.vector.tensor_mul(
                                e[:, 512 + off:512 + off + 128],
                                e[:, 512 + off:512 + off + 128], tri,
                            )
                        nc.tensor.matmul(
                            y0[:, off:512],
                            lhsT=va0[:, 66 * kt:66 * kt + 65],
                            rhs=e[:, off:512],
                            start=(kt == 0), stop=(kt == nkts - 1),
                        )
                        nc.tensor.matmul(
                            y1[:, off:512],
                            lhsT=va1[:, 128 * kt:128 * (kt + 1)],
                            rhs=e[:, 512 + off:1024],
                            start=(kt == 0), stop=(kt == nkts - 1),
                        )

                    # ---- normalize: yn[0:64]=y_h0/den0, yn[64:128]=y_h1/den1
                    yn = ynp.tile([128, 512], BF16)
                    ysb0 = nstg.tile([65, 512], F32, tag="ysb0")
                    nc.scalar.copy(ysb0, y0)
                    ysb1 = nstg.tile([128, 512], F32, tag="ysb1")
                    nc.scalar.copy(ysb1[32:33, :], y1[32:33, :])
                    nc.scalar.copy(ysb1[64:128, :], y1[64:128, :])
                    nc.vector.reciprocal(ysb0[64:65, :], ysb0[64:65, :])
                    nc.vector.reciprocal(ysb1[32:33, :], ysb1[32:33, :])
                    d0 = drp.tile([1, 512], F32, tag="d0")
                    d1 = drp.tile([1, 512], F32, tag="d1")
                    nc.sync.dma_start(d0, ysb0[64:65, :])
                    nc.sync.dma_start(d1, ysb1[32:33, :])
                    rb0 = nstg.tile([64, 512], F32, tag="rb0")
                    nc.gpsimd.dma_start(
                        out=rb0, in_=_bcast_part(d0[0:1, :], 64)
                    )
                    rb1 = nstg.tile([128, 512], F32, tag="rb1")
                    nc.gpsimd.dma_start(
                        out=rb1[64:128, :], in_=_bcast_part(d1[0:1, :], 64)
                    )
                    nc.vector.tensor_mul(yn[0:64, :], ysb0[0:64, :], rb0)
                    nc.vector.tensor_mul(
                        yn[64:128, :], ysb1[64:128, :], rb1[64:128, :]
                    )

                    # ---- projection: outp^T chunk [C, 512] ----
                    for oc in range(C // 128):
                        pp = mmps.tile([128, 512], F32, tag="mm")
                        nc.tensor.matmul(
                            pp,
                            lhsT=wp_sb[:, 128 * oc:128 * (oc + 1)],
                            rhs=yn,
                            start=True, stop=True,
                        )
                        ost = ostg.tile([128, 512], F32)
                        if oc % 2 == 0:
                            nc.vector.tensor_copy(ost, pp)
                        else:
                            nc.scalar.copy(ost, pp)
                        nc.sync.dma_start(
                            outp[128 * oc:128 * (oc + 1), t0 + q0:t0 + q0 + 512],
                            ost,
                        )
    _split_excess_waits(nc)
    return nc


def host_prep(x, w_attn, w_proj, nb=B, tb=T):
    """Slice/cast/transpose inputs per core. Returns in_maps for SPMD."""
    ntok = nb * tb
    x = np.asarray(x, dtype=np.float32).reshape(ntok, C)
    w_attn = np.asarray(w_attn, dtype=np.float32)
    w_proj = np.asarray(w_proj, dtype=np.float32)
    xt = np.ascontiguousarray(x.T).astype(ml_dtypes.bfloat16)
    tri_m = np.triu(np.ones((128, 128), np.float32)).astype(ml_dtypes.bfloat16)
    ident_m = np.eye(128, dtype=np.float32).astype(ml_dtypes.bfloat16)
    in_maps = []
    for s in range(NCORES):
        r0 = DC * s
        wq = w_attn[r0:r0 + DC, :]
        wk = w_attn[C + r0:C + r0 + DC, :]
        wv = w_attn[2 * C + r0:2 * C + r0 + DC, :]
        wqkv_t = np.ascontiguousarray(
            np.concatenate([wq, wk, wv], axis=0).T
        ).astype(ml_dtypes.bfloat16)                       # [C, 384]
        wp_t = np.ascontiguousarray(w_proj[:, r0:r0 + DC].T).astype(
            ml_dtypes.bfloat16
        )                                                  # [128, C]
        in_maps.append({
            "xt": xt, "wqkv": wqkv_t, "wp": wp_t,
            "tri": tri_m, "ident": ident_m,
        })
    return in_maps


def kernel(x, w_attn, w_proj):
    _patch_tile_drain()
    in_maps = host_prep(x, w_attn, w_proj)
    nc = build_attention_nc()
    res = run_bass_kernel_spmd(nc, in_maps, list(range(NCORES)))
    acc = res.results[0]["outp"].astype(np.float32)
    for r in res.results[1:]:
        acc += r["outp"]
    return np.ascontiguousarray(acc.T).reshape(B, T, C).astype(np.float32)


# revision 15
# speedup vs baseline: 1.1135x; 1.1135x over previous
"""Causal self-attention (B=4, T=2048, C=1024, H=16) on 8 TRN2 NeuronCores.

Tensor-parallel over heads: each core owns 2 heads (q/k/v column slice of
c_attn, matching row slice of c_proj) and computes a partial projection
output over the full batch; partials are summed on the host.

Device layout notes (per core):
  - x is staged transposed+bf16 on the host: xt [C, B*T].
  - QKV matmuls produce qT/kT [128, T] per batch (2 heads stacked 64+64 on
    partitions, head_dim on partitions) and a vT staging tile that is
    PE-transposed into token-major v_aug tiles carrying an extra ones
    column (yields the softmax denominator for free during the AV matmul).
  - Scores are computed transposed, S^T [k_tok, q_tok], two heads packed
    into one PE pass via K=64 row-tiling. exp(s/8) runs on ACT straight
    from PSUM into SBUF (no max-subtraction: scores are O(1) here, exp is
    overflow-safe). Causality = trimmed matmul ranges + one static
    triangular mask on diagonal 128-blocks.
  - AV accumulates Y^T in PSUM: head0 as [y(64) | denom] rows 0..64,
    head1 via a [zeros|ones|v] weight layout so its rows land on
    partitions 63..128 (denom row 63) — the two normalized halves form a
    single [128, 512] rhs for the K=128 projection matmul.
  - Normalization: in-place reciprocal of the denom row, DMA
    partition-broadcast through a DRAM bounce, one tensor_mul per head.
  - tri/ident mask constants ship from the host as tiny extra inputs.
Output is the transposed partial projection outp [C, B*T] f32.
"""

import numpy as np
import ml_dtypes

import concourse.bass as bass
import concourse.tile as tile
from concourse import mybir
from concourse.bass_utils import run_bass_kernel_spmd
from concourse.vector_clock import ScopedClock, VectorClock

BF16 = mybir.dt.bfloat16
F32 = mybir.dt.float32
EXPF = mybir.ActivationFunctionType.Exp

B, T, C, H = 4, 2048, 1024, 16
D = C // H          # 64
NCORES = 8
HPC = H // NCORES   # 2 heads per core
DC = HPC * D        # 128 channels per core
SCALE = 1.0 / float(np.sqrt(D))


def _patch_tile_drain():
    """walrus's Drain template rejects >2 sync waits; split the tail-drain
    waits one proc per drain."""
    if getattr(tile.TileContext, "_drain_patched", False):
        return

    def _drain_and_barrier(self, tick_clock, wait_clock):
        nc = self.nc
        gc = tick_clock.global_clock
        n = len(gc)
        for p in range(n):
            if gc[p] > 0:
                d = nc.sync.drain()
                vc = VectorClock([gc[p] if i == p else 0 for i in range(n)])
                wait_clock.add_sem_waits(d.ins, ScopedClock({None: vc}))
        nc.sync.drain()
        nc.all_engine_barrier()
        assert self.sems is not None
        popped = nc._tile_sem_poison_stack.pop()
        assert popped is self._sem_poison
        nc.clear_and_free_semaphores(list(self.sems.allocated().values()))
        nc.all_engine_barrier()

    tile.TileContext._drain_and_barrier = _drain_and_barrier
    tile.TileContext._drain_patched = True


def _split_excess_waits(nc, max_waits=1):
    """walrus's per-instruction sync-wait slot limit is 2; hoist excess
    waits onto same-engine nops inserted just before the instruction."""
    import bass_rust

    snapshots = [
        (bb, list(bb.instructions)) for f in nc.m.functions for bb in f.blocks
    ]
    for bb, insts in snapshots:
        new_list = []
        changed = False
        for inst in insts:
            si = inst.sync_info
            waits = list(si.on_wait) if (si and si.on_wait) else []
            if len(waits) > max_waits:
                changed = True
                excess, keep = waits[:-max_waits], waits[-max_waits:]
                eng = nc.engines[inst.engine]
                for i in range(0, len(excess), max_waits):
                    nop_inst = eng.nop().ins
                    nop_inst.sync_info = bass_rust.SyncInfo(
                        on_wait=list(excess[i:i + max_waits]), on_update=[]
                    )
                    new_list.append(nop_inst)
                inst.sync_info = bass_rust.SyncInfo(
                    on_wait=keep,
                    on_update=list(si.on_update) if si.on_update else [],
                )
            new_list.append(inst)
        bb.instructions = new_list


def _bcast_part(src_row: bass.AP, nparts: int) -> bass.AP:
    """AP reading one partition row broadcast across `nparts` partitions
    (partition stride 0) — used as a DMA source."""
    return bass.AP(
        tensor=src_row.tensor,
        offset=src_row.offset,
        ap=[[0, nparts]] + list(src_row.ap[1:]),
    )


def build_attention_nc(nb: int = B, tb: int = T):
    """One-core program; SPMD across cores via per-core input values."""
    assert tb % 512 == 0
    ntok = nb * tb
    nqc = tb // 512          # q chunks per batch
    nkt = tb // 128          # k tiles per batch
    ncc = C // 128           # contraction chunks for QKV

    nc = bass.Bass("TRN2", target_bir_lowering=False, debug=False)
    xt = nc.dram_tensor("xt", [C, ntok], BF16, kind="ExternalInput").ap()
    wqkv = nc.dram_tensor("wqkv", [C, 3 * DC], BF16, kind="ExternalInput").ap()
    wp = nc.dram_tensor("wp", [DC, C], BF16, kind="ExternalInput").ap()
    trid = nc.dram_tensor("tri", [128, 128], BF16, kind="ExternalInput").ap()
    identd = nc.dram_tensor("ident", [128, 128], BF16, kind="ExternalInput").ap()
    outp = nc.dram_tensor("outp", [C, ntok], F32, kind="ExternalOutput").ap()

    with tile.TileContext(nc) as tc:
        with (
            tc.tile_pool(name="const", bufs=1) as const,
            tc.tile_pool(name="xtp", bufs=2 * ncc) as xtp,
            tc.tile_pool(name="qkp", bufs=2) as qkp,
            tc.tile_pool(name="vap", bufs=2) as vap,
            tc.tile_pool(name="ep", bufs=4) as ep,
            tc.tile_pool(name="nstg", bufs=4) as nstg,
            tc.tile_pool(name="ynp", bufs=2) as ynp,
            tc.tile_pool(name="ostg", bufs=4) as ostg,
            tc.tile_pool(name="drp", bufs=4, space="DRAM") as drp,
            tc.tile_pool(name="mmps", bufs=2, space="PSUM") as mmps,
            tc.tile_pool(name="scps", bufs=2, space="PSUM") as scps,
            tc.tile_pool(name="y0ps", bufs=1, space="PSUM") as y0ps,
            tc.tile_pool(name="y1ps", bufs=1, space="PSUM") as y1ps,
        ):
            # ---- constants ----
            wqkv_sb = const.tile([128, ncc * 3 * DC], BF16)
            for ci in range(ncc):
                nc.gpsimd.dma_start(
                    out=wqkv_sb[:, ci * 3 * DC:(ci + 1) * 3 * DC],
                    in_=wqkv[128 * ci:128 * (ci + 1), :],
                )
            wp_sb = const.tile([128, C], BF16)
            nc.gpsimd.dma_start(out=wp_sb, in_=wp)
            ident = const.tile([128, 128], BF16)
            nc.gpsimd.dma_start(out=ident, in_=identd)
            tri = const.tile([128, 128], BF16)
            nc.gpsimd.dma_start(out=tri, in_=trid)

            for b in range(nb):
                t0 = b * tb
                # ---- load x^T chunks for this batch ----
                xts = []
                for ci in range(ncc):
                    xtile = xtp.tile([128, tb], BF16)
                    nc.gpsimd.dma_start(
                        out=xtile, in_=xt[128 * ci:128 * (ci + 1), t0:t0 + tb]
                    )
                    xts.append(xtile)

                # ---- QKV: qT/kT [128, tb], vT staging ----
                qT = qkp.tile([128, tb], BF16, tag="qT")
                kT = qkp.tile([128, tb], BF16, tag="kT")
                vTs = qkp.tile([128, tb], BF16, tag="vTs")
                for oi, dest in ((0, qT), (1, kT), (2, vTs)):
                    for t4 in range(tb // 512):
                        ps = mmps.tile([128, 512], F32, tag="mm")
                        for ci in range(ncc):
                            nc.tensor.matmul(
                                ps,
                                lhsT=wqkv_sb[
                                    :, ci * 3 * DC + oi * DC:
                                    ci * 3 * DC + (oi + 1) * DC
                                ],
                                rhs=xts[ci][:, 512 * t4:512 * (t4 + 1)],
                                start=(ci == 0),
                                stop=(ci == ncc - 1),
                            )
                        nc.vector.tensor_copy(
                            dest[:, 512 * t4:512 * (t4 + 1)], ps
                        )

                # ---- v_aug: token-major v with ones column ----
                # head0: per ktile 66 cols = [v(64) | 1 | pad]
                # head1: per ktile 128 cols = [0(32) | 1 | 0(31) | v(64)]
                va0 = vap.tile([128, nkt * 66], BF16, tag="va0")
                va1 = vap.tile([128, nkt * 128], BF16, tag="va1")
                va0v = va0.rearrange("p (t c) -> p t c", c=66)
                va1v = va1.rearrange("p (t c) -> p t c", c=128)
                nc.vector.memset(va1, 0.0)
                nc.vector.memset(va0v[:, :, 64:66], 0.0)
                nc.vector.memset(va0v[:, :, 64:65], 1.0)
                nc.vector.memset(va1v[:, :, 32:33], 1.0)
                for g in range(nkt // 4):
                    tps = mmps.tile([128, 512], BF16, tag="mm")
                    for j in range(4):
                        nc.tensor.transpose(
                            tps[:, 128 * j:128 * (j + 1)],
                            vTs[:, 128 * (4 * g + j):128 * (4 * g + j + 1)],
                            ident,
                        )
                    tpsv = tps.rearrange("p (t c) -> p t c", c=128)
                    nc.vector.tensor_copy(
                        va0v[:, 4 * g:4 * g + 4, 0:64], tpsv[:, 0:4, 0:64]
                    )
                    nc.vector.tensor_copy(
                        va1v[:, 4 * g:4 * g + 4, 64:128], tpsv[:, 0:4, 64:128]
                    )

                # ---- attention + projection, per 512-token q chunk ----
                for qc in range(nqc):
                    q0 = 512 * qc
                    y0 = y0ps.tile([65, 512], F32)
                    y1 = y1ps.tile([128, 512], F32)
                    nkts = 4 * (qc + 1)
                    for kt in range(nkts):
                        off = max(0, 128 * kt - q0)
                        sc = scps.tile([128, 1024], F32, tag="sc")
                        nc.tensor.matmul(
                            sc[:, off:512],
                            lhsT=kT[0:64, 128 * kt:128 * (kt + 1)],
                            rhs=qT[0:64, q0 + off:q0 + 512],
                            start=True, stop=True,
                        )
                        nc.tensor.matmul(
                            sc[:, 512 + off:1024],
                            lhsT=kT[64:128, 128 * kt:128 * (kt + 1)],
                            rhs=qT[64:128, q0 + off:q0 + 512],
                            start=True, stop=True,
                        )
                        e = ep.tile([128, 1024], BF16)
                        if off == 0:
                            nc.scalar.activation(e, sc, EXPF, scale=SCALE)
                        else:
                            nc.scalar.activation(
                                e[:, off:512], sc[:, off:512], EXPF,
                                scale=SCALE,
                            )
                            nc.scalar.activation(
                                e[:, 512 + off:1024], sc[:, 512 + off:1024],
                                EXPF, scale=SCALE,
                            )
                        if 128 * kt >= q0:  # diagonal block: triangular mask
                            nc.vector.tensor_mul(
                                e[:, off:off + 128], e[:, off:off + 128], tri
                            )
                            nc.vector.tensor_mul(
                                e[:, 512 + off:512 + off + 128],
                                e[:, 512 + off:512 + off + 128], tri,
                            )
                        nc.tensor.matmul(
                            y0[:, off:512],
                            lhsT=va0[:, 66 * kt:66 * kt + 65],
                            rhs=e[:, off:512],
                            start=(kt == 0), stop=(kt == nkts - 1),
                        )
                        nc.tensor.matmul(
                            y1[:, off:512],
                            lhsT=va1[:, 128 * kt:128 * (kt + 1)],
                            rhs=e[:, 512 + off:1024],
                            start=(kt == 0), stop=(kt == nkts - 1),
                        )

                    # ---- normalize: yn[0:64]=y_h0/den0, yn[64:128]=y_h1/den1
                    yn = ynp.tile([128, 512], BF16)
                    ysb0 = nstg.tile([65, 512], F32, tag="ysb0")
                    nc.scalar.copy(ysb0, y0)
                    ysb1 = nstg.tile([128, 512], F32, tag="ysb1")
                    nc.scalar.copy(ysb1[32:33, :], y1[32:33, :])
                    nc.scalar.copy(ysb1[64:128, :], y1[64:128, :])
                    # denominators: bounce through DRAM to spread the 512
                    # values over 64 DVE lanes (reciprocal is ~8 cyc/elem
                    # serial per lane), recip once for both heads, bounce
                    # back, then partition-broadcast.
                    d0 = drp.tile([1, 512], F32, tag="d0")
                    d1 = drp.tile([1, 512], F32, tag="d1")
                    nc.sync.dma_start(d0, ysb0[64:65, :])
                    nc.sync.dma_start(d1, ysb1[32:33, :])
                    d0v = d0.rearrange("o (p f) -> (o p) f", f=8)
                    d1v = d1.rearrange("o (p f) -> (o p) f", f=8)
                    rs = nstg.tile([64, 16], F32, tag="rs")
                    nc.sync.dma_start(rs[:, 0:8], d0v)
                    nc.sync.dma_start(rs[:, 8:16], d1v)
                    nc.vector.reciprocal(rs, rs)
                    nc.sync.dma_start(d0v, rs[:, 0:8])
                    nc.sync.dma_start(d1v, rs[:, 8:16])
                    rb0 = nstg.tile([64, 512], F32, tag="rb0")
                    nc.gpsimd.dma_start(
                        out=rb0, in_=_bcast_part(d0[0:1, :], 64)
                    )
                    rb1 = nstg.tile([128, 512], F32, tag="rb1")
                    nc.gpsimd.dma_start(
                        out=rb1[64:128, :], in_=_bcast_part(d1[0:1, :], 64)
                    )
                    nc.vector.tensor_mul(yn[0:64, :], ysb0[0:64, :], rb0)
                    nc.vector.tensor_mul(
                        yn[64:128, :], ysb1[64:128, :], rb1[64:128, :]
                    )

                    # ---- projection: outp^T chunk [C, 512] ----
                    for oc in range(C // 128):
                        pp = mmps.tile([128, 512], F32, tag="mm")
                        nc.tensor.matmul(
                            pp,
                            lhsT=wp_sb[:, 128 * oc:128 * (oc + 1)],
                            rhs=yn,
                            start=True, stop=True,
                        )
                        ost = ostg.tile([128, 512], F32)
                        if oc % 2 == 0:
                            nc.vector.tensor_copy(ost, pp)
                        else:
                            nc.scalar.copy(ost, pp)
                        nc.sync.dma_start(
                            outp[128 * oc:128 * (oc + 1), t0 + q0:t0 + q0 + 512],
                            ost,
                        )
    _split_excess_waits(nc)
    return nc


def host_prep(x, w_attn, w_proj, nb=B, tb=T):
    """Slice/cast/transpose inputs per core. Returns in_maps for SPMD."""
    ntok = nb * tb
    x = np.asarray(x, dtype=np.float32).reshape(ntok, C)
    w_attn = np.asarray(w_attn, dtype=np.float32)
    w_proj = np.asarray(w_proj, dtype=np.float32)
    xt = np.ascontiguousarray(x.T).astype(ml_dtypes.bfloat16)
    tri_m = np.triu(np.ones((128, 128), np.float32)).astype(ml_dtypes.bfloat16)
    ident_m = np.eye(128, dtype=np.float32).astype(ml_dtypes.bfloat16)
    in_maps = []
    for s in range(NCORES):
        r0 = DC * s
        wq = w_attn[r0:r0 + DC, :]
        wk = w_attn[C + r0:C + r0 + DC, :]
        wv = w_attn[2 * C + r0:2 * C + r0 + DC, :]
        wqkv_t = np.ascontiguousarray(
            np.concatenate([wq, wk, wv], axis=0).T
        ).astype(ml_dtypes.bfloat16)                       # [C, 384]
        wp_t = np.ascontiguousarray(w_proj[:, r0:r0 + DC].T).astype(
            ml_dtypes.bfloat16
        )                                                  # [128, C]
        in_maps.append({
            "xt": xt, "wqkv": wqkv_t, "wp": wp_t,
            "tri": tri_m, "ident": ident_m,
        })
    return in_maps


def kernel(x, w_attn, w_proj):
    _patch_tile_drain()
    in_maps = host_prep(x, w_attn, w_proj)
    nc = build_attention_nc()
    res = run_bass_kernel_spmd(nc, in_maps, list(range(NCORES)))
    acc = res.results[0]["outp"].astype(np.float32)
    for r in res.results[1:]:
        acc += r["outp"]
    return np.ascontiguousarray(acc.T).reshape(B, T, C).astype(np.float32)


# revision 16
# speedup vs baseline: 1.1682x; 1.0491x over previous
"""Causal self-attention (B=4, T=2048, C=1024, H=16) on 8 TRN2 NeuronCores.

Tensor-parallel over heads: each core owns 2 heads (q/k/v column slice of
c_attn, matching row slice of c_proj) and computes a partial projection
output over the full batch; partials are summed on the host.

Device layout notes (per core):
  - x is staged transposed+bf16 on the host: xt [C, B*T].
  - QKV matmuls produce qT/kT [128, T] per batch (2 heads stacked 64+64 on
    partitions, head_dim on partitions) and a vT staging tile that is
    PE-transposed into token-major v_aug tiles carrying an extra ones
    column (yields the softmax denominator for free during the AV matmul).
  - Scores are computed transposed, S^T [k_tok, q_tok], two heads packed
    into one PE pass via K=64 row-tiling. exp(s/8) runs on ACT straight
    from PSUM into SBUF (no max-subtraction: scores are O(1) here, exp is
    overflow-safe). Causality = trimmed matmul ranges + one static
    triangular mask on diagonal 128-blocks.
  - AV accumulates Y^T in PSUM: head0 as [y(64) | denom] rows 0..64,
    head1 via a [zeros|ones|v] weight layout so its rows land on
    partitions 63..128 (denom row 63) — the two normalized halves form a
    single [128, 512] rhs for the K=128 projection matmul.
  - Normalization: in-place reciprocal of the denom row, DMA
    partition-broadcast through a DRAM bounce, one tensor_mul per head.
  - tri/ident mask constants ship from the host as tiny extra inputs.
Output is the transposed partial projection outp [C, B*T] f32.
"""

import numpy as np
import ml_dtypes

import concourse.bass as bass
import concourse.tile as tile
from concourse import mybir
from concourse.bass_utils import run_bass_kernel_spmd
from concourse.vector_clock import ScopedClock, VectorClock

BF16 = mybir.dt.bfloat16
F32 = mybir.dt.float32
EXPF = mybir.ActivationFunctionType.Exp

B, T, C, H = 4, 2048, 1024, 16
D = C // H          # 64
NCORES = 8
HPC = H // NCORES   # 2 heads per core
DC = HPC * D        # 128 channels per core
SCALE = 1.0 / float(np.sqrt(D))


def _patch_tile_drain():
    """walrus's Drain template rejects >2 sync waits; split the tail-drain
    waits one proc per drain."""
    if getattr(tile.TileContext, "_drain_patched", False):
        return

    def _drain_and_barrier(self, tick_clock, wait_clock):
        nc = self.nc
        gc = tick_clock.global_clock
        n = len(gc)
        for p in range(n):
            if gc[p] > 0:
                d = nc.sync.drain()
                vc = VectorClock([gc[p] if i == p else 0 for i in range(n)])
                wait_clock.add_sem_waits(d.ins, ScopedClock({None: vc}))
        nc.sync.drain()
        nc.all_engine_barrier()
        assert self.sems is not None
        popped = nc._tile_sem_poison_stack.pop()
        assert popped is self._sem_poison
        nc.clear_and_free_semaphores(list(self.sems.allocated().values()))
        nc.all_engine_barrier()

    tile.TileContext._drain_and_barrier = _drain_and_barrier
    tile.TileContext._drain_patched = True


def _split_excess_waits(nc, max_waits=1):
    """walrus's per-instruction sync-wait slot limit is 2; hoist excess
    waits onto same-engine nops inserted just before the instruction."""
    import bass_rust

    snapshots = [
        (bb, list(bb.instructions)) for f in nc.m.functions for bb in f.blocks
    ]
    for bb, insts in snapshots:
        new_list = []
        changed = False
        for inst in insts:
            si = inst.sync_info
            waits = list(si.on_wait) if (si and si.on_wait) else []
            if len(waits) > max_waits:
                changed = True
                excess, keep = waits[:-max_waits], waits[-max_waits:]
                eng = nc.engines[inst.engine]
                for i in range(0, len(excess), max_waits):
                    nop_inst = eng.nop().ins
                    nop_inst.sync_info = bass_rust.SyncInfo(
                        on_wait=list(excess[i:i + max_waits]), on_update=[]
                    )
                    new_list.append(nop_inst)
                inst.sync_info = bass_rust.SyncInfo(
                    on_wait=keep,
                    on_update=list(si.on_update) if si.on_update else [],
                )
            new_list.append(inst)
        bb.instructions = new_list


def _bcast_part(src_row: bass.AP, nparts: int) -> bass.AP:
    """AP reading one partition row broadcast across `nparts` partitions
    (partition stride 0) — used as a DMA source."""
    return bass.AP(
        tensor=src_row.tensor,
        offset=src_row.offset,
        ap=[[0, nparts]] + list(src_row.ap[1:]),
    )


def build_attention_nc(nb: int = B, tb: int = T):
    """One-core program; SPMD across cores via per-core input values."""
    assert tb % 512 == 0
    ntok = nb * tb
    nqc = tb // 512          # q chunks per batch
    nkt = tb // 128          # k tiles per batch
    ncc = C // 128           # contraction chunks for QKV

    nc = bass.Bass("TRN2", target_bir_lowering=False, debug=False)
    xt = nc.dram_tensor("xt", [C, ntok], BF16, kind="ExternalInput").ap()
    wqkv = nc.dram_tensor("wqkv", [C, 3 * DC], BF16, kind="ExternalInput").ap()
    wp = nc.dram_tensor("wp", [DC, C], BF16, kind="ExternalInput").ap()
    trid = nc.dram_tensor("tri", [128, 128], BF16, kind="ExternalInput").ap()
    identd = nc.dram_tensor("ident", [128, 128], BF16, kind="ExternalInput").ap()
    outp = nc.dram_tensor("outp", [C, ntok], F32, kind="ExternalOutput").ap()

    with tile.TileContext(nc) as tc:
        with (
            tc.tile_pool(name="const", bufs=1) as const,
            tc.tile_pool(name="xtp", bufs=2 * ncc) as xtp,
            tc.tile_pool(name="qkp", bufs=2) as qkp,
            tc.tile_pool(name="vap", bufs=2) as vap,
            tc.tile_pool(name="ep", bufs=4) as ep,
            tc.tile_pool(name="nstg", bufs=4) as nstg,
            tc.tile_pool(name="ynp", bufs=2) as ynp,
            tc.tile_pool(name="ostg", bufs=4) as ostg,
            tc.tile_pool(name="drp", bufs=4, space="DRAM") as drp,
            tc.tile_pool(name="mmps", bufs=2, space="PSUM") as mmps,
            tc.tile_pool(name="scps", bufs=2, space="PSUM") as scps,
            tc.tile_pool(name="y0ps", bufs=1, space="PSUM") as y0ps,
            tc.tile_pool(name="y1ps", bufs=1, space="PSUM") as y1ps,
        ):
            # ---- constants ----
            wqkv_sb = const.tile([128, ncc * 3 * DC], BF16)
            for ci in range(ncc):
                nc.gpsimd.dma_start(
                    out=wqkv_sb[:, ci * 3 * DC:(ci + 1) * 3 * DC],
                    in_=wqkv[128 * ci:128 * (ci + 1), :],
                )
            wp_sb = const.tile([128, C], BF16)
            nc.gpsimd.dma_start(out=wp_sb, in_=wp)
            ident = const.tile([128, 128], BF16)
            nc.gpsimd.dma_start(out=ident, in_=identd)
            tri = const.tile([128, 128], BF16)
            nc.gpsimd.dma_start(out=tri, in_=trid)

            for b in range(nb):
                t0 = b * tb
                # ---- load x^T chunks for this batch ----
                xts = []
                for ci in range(ncc):
                    xtile = xtp.tile([128, tb], BF16)
                    nc.gpsimd.dma_start(
                        out=xtile, in_=xt[128 * ci:128 * (ci + 1), t0:t0 + tb]
                    )
                    xts.append(xtile)

                # ---- QKV: qT/kT [128, tb], vT staging ----
                qT = qkp.tile([128, tb], BF16, tag="qT")
                kT = qkp.tile([128, tb], BF16, tag="kT")
                vTs = qkp.tile([128, tb], BF16, tag="vTs")
                for oi, dest in ((0, qT), (1, kT), (2, vTs)):
                    for t4 in range(tb // 512):
                        ps = mmps.tile([128, 512], F32, tag="mm")
                        for ci in range(ncc):
                            nc.tensor.matmul(
                                ps,
                                lhsT=wqkv_sb[
                                    :, ci * 3 * DC + oi * DC:
                                    ci * 3 * DC + (oi + 1) * DC
                                ],
                                rhs=xts[ci][:, 512 * t4:512 * (t4 + 1)],
                                start=(ci == 0),
                                stop=(ci == ncc - 1),
                            )
                        nc.vector.tensor_copy(
                            dest[:, 512 * t4:512 * (t4 + 1)], ps
                        )

                # ---- v_aug: token-major v with ones column ----
                # head0: per ktile 66 cols = [v(64) | 1 | pad]
                # head1: per ktile 128 cols = [0(32) | 1 | 0(31) | v(64)]
                va0 = vap.tile([128, nkt * 66], BF16, tag="va0")
                va1 = vap.tile([128, nkt * 128], BF16, tag="va1")
                va0v = va0.rearrange("p (t c) -> p t c", c=66)
                va1v = va1.rearrange("p (t c) -> p t c", c=128)
                nc.vector.memset(va1, 0.0)
                nc.vector.memset(va0v[:, :, 64:66], 0.0)
                nc.vector.memset(va0v[:, :, 64:65], 1.0)
                nc.vector.memset(va1v[:, :, 32:33], 1.0)
                for g in range(nkt // 4):
                    tps = mmps.tile([128, 512], BF16, tag="mm")
                    for j in range(4):
                        nc.tensor.transpose(
                            tps[:, 128 * j:128 * (j + 1)],
                            vTs[:, 128 * (4 * g + j):128 * (4 * g + j + 1)],
                            ident,
                        )
                    tpsv = tps.rearrange("p (t c) -> p t c", c=128)
                    nc.vector.tensor_copy(
                        va0v[:, 4 * g:4 * g + 4, 0:64], tpsv[:, 0:4, 0:64]
                    )
                    nc.vector.tensor_copy(
                        va1v[:, 4 * g:4 * g + 4, 64:128], tpsv[:, 0:4, 64:128]
                    )

                # ---- attention + projection, per 512-token q chunk ----
                for qc in range(nqc):
                    q0 = 512 * qc
                    y0 = y0ps.tile([65, 512], F32)
                    y1 = y1ps.tile([128, 512], F32)
                    nkts = 4 * (qc + 1)
                    for kt in range(nkts):
                        off = max(0, 128 * kt - q0)
                        sc = scps.tile([128, 1024], F32, tag="sc")
                        nc.tensor.matmul(
                            sc[:, off:512],
                            lhsT=kT[0:64, 128 * kt:128 * (kt + 1)],
                            rhs=qT[0:64, q0 + off:q0 + 512],
                            start=True, stop=True,
                        )
                        nc.tensor.matmul(
                            sc[:, 512 + off:1024],
                            lhsT=kT[64:128, 128 * kt:128 * (kt + 1)],
                            rhs=qT[64:128, q0 + off:q0 + 512],
                            start=True, stop=True,
                        )
                        e = ep.tile([128, 1024], BF16)
                        if off == 0:
                            nc.scalar.activation(e, sc, EXPF, scale=SCALE)
                        else:
                            nc.scalar.activation(
                                e[:, off:512], sc[:, off:512], EXPF,
                                scale=SCALE,
                            )
                            nc.scalar.activation(
                                e[:, 512 + off:1024], sc[:, 512 + off:1024],
                                EXPF, scale=SCALE,
                            )
                        if 128 * kt >= q0:  # diagonal block: triangular mask
                            nc.vector.tensor_mul(
                                e[:, off:off + 128], e[:, off:off + 128], tri
                            )
                            nc.vector.tensor_mul(
                                e[:, 512 + off:512 + off + 128],
                                e[:, 512 + off:512 + off + 128], tri,
                            )
                        nc.tensor.matmul(
                            y0[:, off:512],
                            lhsT=va0[:, 66 * kt:66 * kt + 65],
                            rhs=e[:, off:512],
                            start=(kt == 0), stop=(kt == nkts - 1),
                        )
                        nc.tensor.matmul(
                            y1[:, off:512],
                            lhsT=va1[:, 128 * kt:128 * (kt + 1)],
                            rhs=e[:, 512 + off:1024],
                            start=(kt == 0), stop=(kt == nkts - 1),
                        )

                    # ---- normalize: yn[0:64]=y_h0/den0, yn[64:128]=y_h1/den1
                    yn = ynp.tile([128, 512], BF16)
                    ysb0 = nstg.tile([65, 512], F32, tag="ysb0")
                    nc.scalar.copy(ysb0, y0)
                    ysb1 = nstg.tile([128, 512], F32, tag="ysb1")
                    nc.vector.tensor_copy(ysb1[32:33, :], y1[32:33, :])
                    nc.vector.tensor_copy(ysb1[64:128, :], y1[64:128, :])
                    # denominators: bounce through DRAM to spread the 512
                    # values over 64 DVE lanes (reciprocal is ~8 cyc/elem
                    # serial per lane), recip once for both heads, bounce
                    # back, then partition-broadcast.
                    d0 = drp.tile([1, 512], F32, tag="d0")
                    d1 = drp.tile([1, 512], F32, tag="d1")
                    nc.sync.dma_start(d0, ysb0[64:65, :])
                    nc.sync.dma_start(d1, ysb1[32:33, :])
                    d0v = d0.rearrange("o (p f) -> (o p) f", f=8)
                    d1v = d1.rearrange("o (p f) -> (o p) f", f=8)
                    rs = nstg.tile([64, 16], F32, tag="rs")
                    nc.sync.dma_start(rs[:, 0:8], d0v)
                    nc.sync.dma_start(rs[:, 8:16], d1v)
                    nc.vector.reciprocal(rs, rs)
                    nc.sync.dma_start(d0v, rs[:, 0:8])
                    nc.sync.dma_start(d1v, rs[:, 8:16])
                    rb0 = nstg.tile([64, 512], F32, tag="rb0")
                    nc.sync.dma_start(rb0, _bcast_part(d0[0:1, :], 64))
                    rb1 = nstg.tile([128, 512], F32, tag="rb1")
                    nc.sync.dma_start(
                        rb1[64:128, :], _bcast_part(d1[0:1, :], 64)
                    )
                    nc.vector.tensor_mul(yn[0:64, :], ysb0[0:64, :], rb0)
                    nc.vector.tensor_mul(
                        yn[64:128, :], ysb1[64:128, :], rb1[64:128, :]
                    )

                    # ---- projection: outp^T chunk [C, 512] ----
                    for oc in range(C // 128):
                        pp = mmps.tile([128, 512], F32, tag="mm")
                        nc.tensor.matmul(
                            pp,
                            lhsT=wp_sb[:, 128 * oc:128 * (oc + 1)],
                            rhs=yn,
                            start=True, stop=True,
                        )
                        ost = ostg.tile([128, 512], F32)
                        nc.vector.tensor_copy(ost, pp)
                        nc.sync.dma_start(
                            outp[128 * oc:128 * (oc + 1), t0 + q0:t0 + q0 + 512],
                            ost,
                        )
    _split_excess_waits(nc)
    return nc


def host_prep(x, w_attn, w_proj, nb=B, tb=T):
    """Slice/cast/transpose inputs per core. Returns in_maps for SPMD."""
    ntok = nb * tb
    x = np.asarray(x, dtype=np.float32).reshape(ntok, C)
    w_attn = np.asarray(w_attn, dtype=np.float32)
    w_proj = np.asarray(w_proj, dtype=np.float32)
    xt = np.ascontiguousarray(x.T).astype(ml_dtypes.bfloat16)
    tri_m = np.triu(np.ones((128, 128), np.float32)).astype(ml_dtypes.bfloat16)
    ident_m = np.eye(128, dtype=np.float32).astype(ml_dtypes.bfloat16)
    in_maps = []
    for s in range(NCORES):
        r0 = DC * s
        wq = w_attn[r0:r0 + DC, :]
        wk = w_attn[C + r0:C + r0 + DC, :]
        wv = w_attn[2 * C + r0:2 * C + r0 + DC, :]
        wqkv_t = np.ascontiguousarray(
            np.concatenate([wq, wk, wv], axis=0).T
        ).astype(ml_dtypes.bfloat16)                       # [C, 384]
        wp_t = np.ascontiguousarray(w_proj[:, r0:r0 + DC].T).astype(
            ml_dtypes.bfloat16
        )                                                  # [128, C]
        in_maps.append({
            "xt": xt, "wqkv": wqkv_t, "wp": wp_t,
            "tri": tri_m, "ident": ident_m,
        })
    return in_maps


def kernel(x, w_attn, w_proj):
    _patch_tile_drain()
    in_maps = host_prep(x, w_attn, w_proj)
    nc = build_attention_nc()
    res = run_bass_kernel_spmd(nc, in_maps, list(range(NCORES)))
    acc = res.results[0]["outp"].astype(np.float32)
    for r in res.results[1:]:
        acc += r["outp"]
    return np.ascontiguousarray(acc.T).reshape(B, T, C).astype(np.float32)
